# revision 1
# baseline (speedup 1.0000x reference)
"""Trainium2 Bass kernel for nn_MessagePassingLayer (gnn_message_passing).

Computes, for x:[B,C,N,1] f32, edge_index:[B,N,K] i32, alpha scalar:
    out[b,c,n] = x[b,c,n]*(1+alpha) + sum_k x[b,c,edge_index[b,n,k]]

Sharding: B=8 batch samples, one per NeuronCore (data parallel). Edge
indices are intra-sample so there is no cross-core communication.

Per-core device program:
  - load node-major table xt [N=4096, C=64] (host-transposed layout)
  - 16x dma_gather (SWDGE row gather from HBM, 4096 rows x 256B each)
  - DVE pairwise-tree accumulation of the 16 gathered tiles
  - out = xt*(1+alpha) + sum  (DVE), stored node-major; host transposes back
"""
import os
import sys
import types

import numpy as np

B, C, N, K = 8, 64, 4096, 16
NCORES = 8
P = 128
COLS = N // P  # 32 nodes per partition
FREE = COLS * C  # 2048 f32 per partition

LAST_EXEC_NS = None


# ---------------------------------------------------------------------------
# axon NTFF profile hook shim (the agent image's antenv lacks axon_hooks)
# ---------------------------------------------------------------------------
def _install_profile_shim():
    if "antenv.axon_hooks" in sys.modules:
        return
    try:
        import antenv

        mod = types.ModuleType("antenv.axon_hooks")
        mod._hook = None
        mod.set_axon_ntff_profile_hook = lambda h: setattr(mod, "_hook", h)
        mod.get_axon_ntff_profile_hook = lambda: mod._hook
        sys.modules["antenv.axon_hooks"] = mod
        antenv.axon_hooks = mod
        from trn_agent_boot.trn_boot import _ntff_profile_via_ctypes

        mod.set_axon_ntff_profile_hook(
            _ntff_profile_via_ctypes("/opt/axon/libaxon_pjrt.so")
        )
    except Exception:
        pass


# ---------------------------------------------------------------------------
# Walrus in this container rejects >1 sync-wait per instruction. Split any
# multi-wait instruction into single-wait NoOps on the same engine.
# ---------------------------------------------------------------------------
def _split_multiwaits(nc, mybir):
    cnt = [0]
    for f in nc.m.functions:
        for bb in f.blocks:
            new_list = []
            for ins in bb.instructions:
                si = ins.sync_info
                if si is not None and si.on_wait and len(si.on_wait) > 1:
                    waits = list(si.on_wait)
                    for w in waits[:-1]:
                        cnt[0] += 1
                        nop = mybir.InstNoOp(name=f"I-waitsplit-{cnt[0]}")
                        nop.engine = ins.engine
                        nop.sync_info = mybir.SyncInfo(on_wait=[w], on_update=[])
                        try:
                            nc.register_instruction(nop, overwrite=True)
                        except Exception:
                            pass
                        new_list.append(nop)
                    ins.sync_info = mybir.SyncInfo(
                        on_wait=[waits[-1]], on_update=list(si.on_update)
                    )
                new_list.append(ins)
            bb.instructions = new_list


# ---------------------------------------------------------------------------
# Device program
# ---------------------------------------------------------------------------
GATHER_CHUNK = int(os.environ.get("KERNEL_GATHER_CHUNK", "2048"))
SCRATCH = int(os.environ.get("KERNEL_SCRATCH", "16384"))
SINGLE_PACKET = bool(int(os.environ.get("KERNEL_SINGLE_PACKET", "0")))


def _build_program():
    import concourse.mybir as mybir
    import concourse.tile as tile
    from concourse import bacc

    nc = bacc.Bacc("TRN2", target_bir_lowering=False, debug=False,
                   num_devices=NCORES, num_swdge_queues=4,
                   dynamic_dma_scratch_size=SCRATCH)
    xt_d = nc.dram_tensor("xt", [N, C], mybir.dt.float32, kind="ExternalInput")
    idx_d = nc.dram_tensor("idx", [P, K * (N // 16)], mybir.dt.int16,
                           kind="ExternalInput")
    alpha_d = nc.dram_tensor("alpha", [P, 1], mybir.dt.float32,
                             kind="ExternalInput")
    out_d = nc.dram_tensor("out", [N, C], mybir.dt.float32,
                           kind="ExternalOutput")

    cpk = N // GATHER_CHUNK      # segments (chunk instructions per k)
    ipg = GATHER_CHUNK // 16     # idx cols per chunk
    opg = GATHER_CHUNK // P      # out free-cols per chunk
    SEGW = opg * C               # f32 cols per segment in node-major tiles

    with tile.TileContext(nc) as tc:
        with tc.tile_pool(name="sbuf", bufs=1) as pool:
            xt_sb = pool.tile([P, FREE], mybir.dt.float32, tag="xt")
            idx_sb = [pool.tile([P, K * ipg], mybir.dt.int16, tag=f"idx{c}",
                                name=f"idx{c}") for c in range(cpk)]
            al_sb = pool.tile([P, 1], mybir.dt.float32, tag="al")
            g = [[pool.tile([P, SEGW], mybir.dt.float32, tag=f"g{k}_{c}",
                            name=f"g{k}_{c}") for c in range(cpk)]
                 for k in range(K)]
            o = [pool.tile([P, SEGW], mybir.dt.float32, tag=f"o{c}",
                           name=f"o{c}") for c in range(cpk)]

            xt_nm = xt_d.ap().rearrange("(p a) c -> p (a c)", p=P)
            out_nm = out_d.ap().rearrange("(p a) c -> p (a c)", p=P)

            # segment-0 idx load first, split across both HWDGE engines so
            # the first gathers start ASAP; later segments' idx tiles are
            # loaded after the first gather wave is issued.
            engs = [nc.sync, nc.scalar]
            kq = K // 2
            for j in range(2):
                engs[j].dma_start(
                    out=idx_sb[0][:, j * kq * ipg:(j + 1) * kq * ipg],
                    in_=idx_d.ap()[:, j * kq * ipg:(j + 1) * kq * ipg],
                )
            # shared register for the (constant) per-gather index count
            nreg = nc.gpsimd.to_reg(GATHER_CHUNK)

            gi = 0
            for c in range(cpk):
                for k in range(K):
                    nc.gpsimd.dma_gather(
                        out_ap=g[k][c][:].rearrange("p (a c) -> p a c", c=C),
                        in_ap=xt_d.ap(),
                        idxs_ap=idx_sb[c][:, k * ipg:(k + 1) * ipg],
                        num_idxs=GATHER_CHUNK,
                        num_idxs_reg=nreg,
                        elem_size=C,
                        queue_num=gi % 4,
                        single_packet=SINGLE_PACKET,
                    )
                    gi += 1
                if c == 0:
                    # remaining idx segments + xt/alpha, issued after the
                    # first gather wave so they never gate gather 0
                    for cc in range(1, cpk):
                        nc.sync.dma_start(
                            out=idx_sb[cc][:],
                            in_=idx_d.ap()[:, cc * K * ipg:
                                           (cc + 1) * K * ipg],
                        )
                    nc.scalar.dma_start(out=al_sb[:], in_=alpha_d.ap())
                    nc.scalar.dma_start(out=xt_sb[:], in_=xt_nm)
                    nc.scalar.add(out=al_sb[:], in_=al_sb[:], add=1.0)

            for c in range(cpk):
                # o = xt*(1+alpha), then running accumulation in gather
                # completion (issue) order so only the last add is tail
                nc.vector.tensor_scalar_mul(
                    out=o[c][:], in0=xt_sb[:, c * SEGW:(c + 1) * SEGW],
                    scalar1=al_sb[:, :1],
                )
                for k in range(K):
                    nc.vector.tensor_add(
                        out=o[c][:], in0=o[c][:], in1=g[k][c][:],
                    )
                nc.sync.dma_start(
                    out=out_nm[:, c * SEGW:(c + 1) * SEGW], in_=o[c][:],
                )

    nc.compile()
    _split_multiwaits(nc, mybir)
    return nc


_PROGRAM = None


def _get_program():
    global _PROGRAM
    if _PROGRAM is None:
        _PROGRAM = _build_program()
    return _PROGRAM


# ---------------------------------------------------------------------------
# Host glue
# ---------------------------------------------------------------------------
_slot = np.arange(N)
_PERM = (_slot % P) * COLS + (_slot // P)  # node id for flat gather slot i


def _prep_idx(edge_b):
    """edge_b [N, K] int32 -> wrapped int16 [128, K*N/16] for dma_gather,
    laid out segment-major: col block (c, k) holds chunk c of gather k."""
    cpk = N // GATHER_CHUNK
    ipg = GATHER_CHUNK // 16
    ids = edge_b[_PERM, :].astype(np.int16)          # [4096 slots, K]
    f = ids.T.reshape(K, N // 16, 16)                # [K, s=256, p16]
    w = np.transpose(f, (2, 0, 1))                   # [p16, K, 256]
    w = np.tile(w, (8, 1, 1))                        # [128, K, 256]
    # wait: chunk c of gather k covers slots [c*CHUNK, (c+1)*CHUNK), i.e.
    # wrapped cols [c*ipg, (c+1)*ipg) of k's block -> reorder to (c, k, ipg)
    w = w.reshape(P, K, cpk, ipg).transpose(0, 2, 1, 3)
    return np.ascontiguousarray(w.reshape(P, K * (N // 16)))


def kernel(x, edge_index, alpha):
    global LAST_EXEC_NS
    _install_profile_shim()
    from concourse import bass_utils

    x = np.asarray(x)
    edge_index = np.asarray(edge_index)
    alpha_v = np.float32(np.asarray(alpha))

    nc = _get_program()

    xt = np.transpose(x[..., 0], (0, 2, 1))  # [B, N, C]
    in_maps = []
    for b in range(B):
        in_maps.append({
            "xt": np.ascontiguousarray(xt[b]),
            "idx": _prep_idx(edge_index[b]),
            "alpha": np.full((P, 1), alpha_v, dtype=np.float32),
        })

    trace = bool(int(os.environ.get("KERNEL_PROFILE", "0")))
    res = bass_utils.run_bass_kernel_spmd(
        nc, in_maps, core_ids=list(range(NCORES)), trace=trace
    )
    LAST_EXEC_NS = res.exec_time_ns

    out = np.empty((B, C, N, 1), dtype=np.float32)
    for b in range(B):
        out[b, :, :, 0] = res.results[b]["out"].T
    return out



# revision 5
# speedup vs baseline: 1.8364x; 1.8364x over previous
"""Trainium2 Bass kernel for nn_MessagePassingLayer (gnn_message_passing).

Computes, for x:[B,C,N,1] f32, edge_index:[B,N,K] i32, alpha scalar:
    out[b,c,n] = x[b,c,n]*(1+alpha) + sum_k x[b,c,edge_index[b,n,k]]

Sharding: B=8 batch samples, one per NeuronCore (data parallel). Edge
indices are intra-sample so there is no cross-core communication.

Mechanism: the neighbor gather+sum is a dense matmul against the
(host-built) adjacency-count matrix:
    m[c, n] = sum_src A[n, src] * x[src, c]  =  (x_nodes^T A^T)^T
TensorE computes it with Aᵀ streamed from HBM as fp8 (counts 0..16 are
exact in e4m3) and x as stationary in two fp8 terms (hi + residual) that
accumulate into the same PSUM region, recovering ~bf16 precision.
This avoids SWDGE dma_gather entirely (the previous approach was capped
by the 4-queue software-DGE descriptor floor at ~134 us/core).

Per-core device program:
  - stream Aᵀ fp8 [4096 src, 4096 dst] in 32 tiles of [128, 2x2048]
    round-robin over 4 HWDGE queues (sync/scalar/vector/gpsimd)
  - 256 DoubleRow fp8 matmuls accumulate A@x_hi + A@x_w into 8 PSUM
    banks of [64 ch, 512 dst]
  - DVE: out = x*(1+alpha) + psum per column group, store channel-major
"""
import os
import sys
import types

import numpy as np

B, C, N, K = 8, 64, 4096, 16
NCORES = 8
P = 128
NPAIR = N // (2 * P)     # 16 contraction pair-blocks (DoubleRow: 256 rows)
NHALF = 2                # dst column halves (pipeline psum banks 0-3 / 4-7)
HCOLS = N // NHALF       # 2048 dst cols per half
GRP = 512                # psum bank free size (f32)
NGRP = HCOLS // GRP      # 4 col groups per half

LAST_EXEC_NS = None


# ---------------------------------------------------------------------------
# axon NTFF profile hook shim (the agent image's antenv lacks axon_hooks)
# ---------------------------------------------------------------------------
def _install_profile_shim():
    if "antenv.axon_hooks" in sys.modules:
        return
    try:
        import antenv

        mod = types.ModuleType("antenv.axon_hooks")
        mod._hook = None
        mod.set_axon_ntff_profile_hook = lambda h: setattr(mod, "_hook", h)
        mod.get_axon_ntff_profile_hook = lambda: mod._hook
        sys.modules["antenv.axon_hooks"] = mod
        antenv.axon_hooks = mod
        from trn_agent_boot.trn_boot import _ntff_profile_via_ctypes

        mod.set_axon_ntff_profile_hook(
            _ntff_profile_via_ctypes("/opt/axon/libaxon_pjrt.so")
        )
    except Exception:
        pass


# ---------------------------------------------------------------------------
# Walrus in this container rejects >1 sync-wait per instruction. Split any
# multi-wait instruction into single-wait NoOps on the same engine.
# ---------------------------------------------------------------------------
def _split_multiwaits(nc, mybir):
    cnt = [0]
    for f in nc.m.functions:
        for bb in f.blocks:
            new_list = []
            for ins in bb.instructions:
                si = ins.sync_info
                if si is not None and si.on_wait and len(si.on_wait) > 1:
                    waits = list(si.on_wait)
                    for w in waits[:-1]:
                        cnt[0] += 1
                        nop = mybir.InstNoOp(name=f"I-waitsplit-{cnt[0]}")
                        nop.engine = ins.engine
                        nop.sync_info = mybir.SyncInfo(on_wait=[w], on_update=[])
                        try:
                            nc.register_instruction(nop, overwrite=True)
                        except Exception:
                            pass
                        new_list.append(nop)
                    ins.sync_info = mybir.SyncInfo(
                        on_wait=[waits[-1]], on_update=list(si.on_update)
                    )
                new_list.append(ins)
            bb.instructions = new_list


# ---------------------------------------------------------------------------
# Device program
# ---------------------------------------------------------------------------
def _build_program():
    import concourse.mybir as mybir
    import concourse.tile as tile
    from concourse import bacc

    nc = bacc.Bacc("TRN2", target_bir_lowering=False, debug=False,
                   num_devices=NCORES)
    # Aᵀ fp8 bytes, laid out [ (h, q, p) , (t, n) ]:
    #   row h*16*128 + q*128 + p, col t*2048 + j  =  A[2048h+j, 256q+128t+p]
    at_d = nc.dram_tensor("at", [N, N], mybir.dt.uint8, kind="ExternalInput")
    # stationary x fp8 bytes: [p, (q, t, s, c)] with s = {hi, w}
    xs_d = nc.dram_tensor("xs", [P, NPAIR * 2 * 2 * C], mybir.dt.uint8,
                          kind="ExternalInput")
    # x channel-major f32 (the (1+alpha)*x term)
    x_d = nc.dram_tensor("x", [C, N], mybir.dt.float32, kind="ExternalInput")
    alpha_d = nc.dram_tensor("alpha", [P, 1], mybir.dt.float32,
                             kind="ExternalInput")
    out_d = nc.dram_tensor("out", [C, N], mybir.dt.float32,
                           kind="ExternalOutput")

    NAT = 4  # rotating Aᵀ SBUF buffers
    fp8 = mybir.dt.float8e4

    with tile.TileContext(nc) as tc:
        with tc.tile_pool(name="sbuf", bufs=1) as pool, \
             tc.tile_pool(name="atp", bufs=NAT) as atp, \
             tc.tile_pool(name="psum", bufs=1, space="PSUM") as ppool:
            xs_sb = pool.tile([P, NPAIR * 2 * 2 * C], mybir.dt.uint8,
                              tag="xs")
            x_sb = pool.tile([C, N], mybir.dt.float32, tag="x")
            al_sb = pool.tile([P, 1], mybir.dt.float32, tag="al")
            o_sb = [pool.tile([C, GRP], mybir.dt.float32, tag=f"o{g}",
                              name=f"o{g}") for g in range(NHALF * NGRP)]
            ps = [ppool.tile([C, GRP], mybir.dt.float32, tag=f"ps{g}",
                             name=f"ps{g}") for g in range(NHALF * NGRP)]

            engs = [nc.sync, nc.scalar, nc.gpsimd]
            NENG = len(engs)

            nc.scalar.dma_start(out=xs_sb[:], in_=xs_d.ap())
            nc.gpsimd.dma_start(out=al_sb[:], in_=alpha_d.ap())
            nc.gpsimd.dma_start(out=x_sb[:], in_=x_d.ap())
            nc.scalar.add(out=al_sb[:], in_=al_sb[:], add=1.0)

            # stationary views: [128, 2, 64] fp8 per (pair q, s)
            xs4 = xs_sb[:].rearrange("p (q t s c) -> p q t s c",
                                     q=NPAIR, t=2, s=2)

            st_eng = 0
            for h in range(NHALF):
                for q in range(NPAIR):
                    i = h * NPAIR + q
                    at_t = atp.tile([P, 2 * HCOLS], mybir.dt.uint8,
                                    tag="at")
                    engs[i % NENG].dma_start(
                        out=at_t[:], in_=at_d.ap()[i * P:(i + 1) * P, :],
                    )
                    at3 = at_t[:].rearrange(
                        "p (t n) -> p t n", t=2).bitcast(fp8)
                    for g in range(NGRP):
                        gi = h * NGRP + g
                        mv = at3[:, :, g * GRP:(g + 1) * GRP]
                        for s in range(2):
                            nc.tensor.matmul(
                                ps[gi][:],
                                xs4[:, q, :, s, :].bitcast(fp8),
                                mv,
                                start=(q == 0 and s == 0),
                                stop=(q == NPAIR - 1 and s == 1),
                                perf_mode=mybir.MatmulPerfMode.DoubleRow,
                            )
                # half h done: combine with (1+alpha)*x and store
                for g in range(NGRP):
                    gi = h * NGRP + g
                    lo = h * HCOLS + g * GRP
                    nc.vector.scalar_tensor_tensor(
                        out=o_sb[gi][:],
                        in0=x_sb[:, lo:lo + GRP],
                        scalar=al_sb[0:C, 0:1],
                        in1=ps[gi][:],
                        op0=mybir.AluOpType.mult,
                        op1=mybir.AluOpType.add,
                    )
                    engs[st_eng % NENG].dma_start(
                        out=out_d.ap()[:, lo:lo + GRP], in_=o_sb[gi][:],
                    )
                    st_eng += 1

    nc.compile()
    _split_multiwaits(nc, mybir)
    return nc


_PROGRAM = None


def _get_program():
    global _PROGRAM
    if _PROGRAM is None:
        _PROGRAM = _build_program()
    return _PROGRAM


# ---------------------------------------------------------------------------
# Host glue
# ---------------------------------------------------------------------------
def _fp8_lut():
    import ml_dtypes

    return np.arange(K + 1).astype(ml_dtypes.float8_e4m3fn).view(np.uint8)


_LUT = None


def _prep_at(edge_b):
    """edge_b [N, K] int32 -> Aᵀ fp8 bytes in the device tile layout."""
    global _LUT
    if _LUT is None:
        _LUT = _fp8_lut()
    src = edge_b.astype(np.int64)                       # [N dst, K]
    flat = (src * N + np.arange(N, dtype=np.int64)[:, None]).ravel()
    cnt = np.bincount(flat, minlength=N * N)            # Aᵀ[src, dst] counts
    at = _LUT[cnt]                                      # uint8 fp8 bytes
    # [src, dst] -> [(h, q, p), (t, n)]
    at5 = at.reshape(NPAIR, 2, P, NHALF, HCOLS)         # (q, t, p, h, n)
    at5 = at5.transpose(3, 0, 2, 1, 4)                  # (h, q, p, t, n)
    return np.ascontiguousarray(at5.reshape(N, N))


def _prep_xs(xt_b):
    """xt_b [N, C] f32 node-major -> stationary fp8 bytes [128, q*t*s*C]."""
    import ml_dtypes

    hi = xt_b.astype(ml_dtypes.float8_e4m3fn)
    w = (xt_b - hi.astype(np.float32)).astype(ml_dtypes.float8_e4m3fn)
    hw = np.stack([hi.view(np.uint8), w.view(np.uint8)], axis=1)  # [N, s, C]
    hw = hw.reshape(NPAIR, 2, P, 2, C)                  # (q, t, p, s, c)
    hw = hw.transpose(2, 0, 1, 3, 4)                    # (p, q, t, s, c)
    return np.ascontiguousarray(hw.reshape(P, NPAIR * 2 * 2 * C))


def kernel(x, edge_index, alpha):
    global LAST_EXEC_NS
    _install_profile_shim()
    from concourse import bass_utils

    x = np.asarray(x)
    edge_index = np.asarray(edge_index)
    alpha_v = np.float32(np.asarray(alpha))

    nc = _get_program()

    in_maps = []
    for b in range(B):
        xt = np.ascontiguousarray(x[b, :, :, 0].T)      # [N, C]
        in_maps.append({
            "at": _prep_at(edge_index[b]),
            "xs": _prep_xs(xt),
            "x": np.ascontiguousarray(x[b, :, :, 0]),
            "alpha": np.full((P, 1), alpha_v, dtype=np.float32),
        })

    trace = bool(int(os.environ.get("KERNEL_PROFILE", "0")))
    res = bass_utils.run_bass_kernel_spmd(
        nc, in_maps, core_ids=list(range(NCORES)), trace=trace
    )
    LAST_EXEC_NS = res.exec_time_ns

    out = np.empty((B, C, N, 1), dtype=np.float32)
    for b in range(B):
        out[b, :, :, 0] = res.results[b]["out"]
    return out
